# revision 1
# baseline (speedup 1.0000x reference)
"""RWKV-5 block (TimeMix + ChannelMix) on 8 Trainium2 NeuronCores.

Sharding: 2 batch groups x 4-way tensor-parallel (core = 4*g + lane).
TimeMix heads split 8/lane; (att*g)^T AllGathered per group, Wo replicated.
ChannelMix FF split 2048/lane; kv partials ReduceScattered by C rows.
Activations channel-major (x^T [C,T]); LN stats via PE ones-reduction;
WKV chunked (L=128) with precomputed decay power tables; matmuls float32r.
Host assembles the full [B,T,C] output from per-core row slices.
"""
import sys
import numpy as np

sys.path.insert(0, '/opt/trn_rl_repo')

B, T, C, H, N, FF = 2, 1024, 2048, 32, 64, 8192
EPS = 1e-5
L = 128            # WKV chunk length
NCH = T // L       # 8 chunks
NCORES = 8
LANES = 4
HPL = H // LANES   # 8 heads per lane
CHL = HPL * N      # 512 att channels per lane
FFL = FF // LANES  # 2048 ff channels per lane
KT = C // 128      # 16 contraction tiles
S = 512            # token free-dim chunk
GROUPS = [[0, 1, 2, 3], [4, 5, 6, 7]]

_PROGRAM = None


def _build_program():
    import concourse.bacc as bacc
    import concourse.tile as tile
    from concourse import mybir
    from contextlib import ExitStack

    F32 = mybir.dt.float32
    F32R = mybir.dt.float32r
    ALU = mybir.AluOpType
    ACT = mybir.ActivationFunctionType

    nc = bacc.Bacc("TRN2", target_bir_lowering=False, debug=False,
                   num_devices=NCORES)

    def din(name, shape):
        return nc.dram_tensor(name, shape, F32, kind="ExternalInput").ap()

    xT = din("xT", [C, T])
    Wr = din("Wr", [C, CHL]); Wk = din("Wk", [C, CHL])
    Wv = din("Wv", [C, CHL]); Wg = din("Wg", [C, CHL])
    Wo = din("Wo", [C, C])
    Wkey = din("Wkey", [C, FFL]); Wval = din("Wval", [FFL, C])
    Wrec = din("Wrec", [C, CHL])
    tmK = din("tmK", [C, 1]); tmV = din("tmV", [C, 1])
    tmR = din("tmR", [C, 1]); tmG = din("tmG", [C, 1])
    fmK = din("fmK", [C, 1]); fmR = din("fmR", [C, 1])
    POW_R = din("POW_R", [CHL, L]); POW_K = din("POW_K", [CHL, L])
    POW_U = din("POW_U", [CHL, L]); POW_CT = din("POW_CT", [L, CHL])
    DL = din("DL", [CHL, 1])
    MASKT = din("MASKT", [L, L]); IDENT = din("IDENT", [L, L])
    ONESC = din("ONESC", [128, 1]); ONESR = din("ONESR", [1, 128])
    ZERO64 = din("ZERO64", [128, 64])

    o1 = nc.dram_tensor("o1", [CHL, T], F32, kind="ExternalOutput").ap()
    x2out = nc.dram_tensor("x2out", [C, T], F32, kind="ExternalOutput").ap()

    cc_in = nc.dram_tensor("cc_in", [CHL, T], F32).ap()
    ag_out = nc.dram_tensor("ag_out", [C, T], F32).ap()
    rs_in = nc.dram_tensor("rs_in", [C, T], F32).ap()
    rs_out = nc.dram_tensor("rs_out", [CHL, T], F32).ap()
    kT_dram = nc.dram_tensor("kT_dram", [CHL, T], F32).ap()
    g_dram = nc.dram_tensor("g_dram", [T, CHL], F32).ap()
    ck_dram = nc.dram_tensor("ck_dram", [C, T], F32).ap()
    rT_dram = nc.dram_tensor("rT_dram", [CHL, T], F32).ap()

    with tile.TileContext(nc) as tc, ExitStack() as ctx:
        csts = ctx.enter_context(tc.tile_pool(name="csts", bufs=1))
        big = ctx.enter_context(tc.tile_pool(name="big", bufs=1))
        rot = ctx.enter_context(tc.tile_pool(name="rot", bufs=3))
        rot2 = ctx.enter_context(tc.tile_pool(name="rot2", bufs=2))
        outs = ctx.enter_context(tc.tile_pool(name="outs", bufs=1))
        wkvp = ctx.enter_context(tc.tile_pool(name="wkvp", bufs=3))
        state = ctx.enter_context(tc.tile_pool(name="state", bufs=2))
        ps_big = ctx.enter_context(
            tc.tile_pool(name="ps_big", bufs=4, space="PSUM"))
        ps_y = ctx.enter_context(tc.tile_pool(name="ps_y", bufs=1, space="PSUM"))
        ps_sd = ctx.enter_context(
            tc.tile_pool(name="ps_sd", bufs=1, space="PSUM"))
        ps_sm = ctx.enter_context(
            tc.tile_pool(name="ps_sm", bufs=2, space="PSUM"))

        # ---------------- constants ----------------
        _cst_n = [0]
        def load_const(ap, shape, rearr=None, dt=F32, p=128):
            _cst_n[0] += 1
            nm = f"cst{_cst_n[0]}"
            t = csts.tile(shape, dt, name=nm, tag=nm)
            src = ap if rearr is None else ap.rearrange(rearr, p=p)
            if dt == F32R:
                src = src.bitcast(F32R)
            nc.sync.dma_start(out=t, in_=src)
            return t

        tmK_t = load_const(tmK, [128, KT], "(kt p) o -> p (kt o)")
        tmV_t = load_const(tmV, [128, KT], "(kt p) o -> p (kt o)")
        tmR_t = load_const(tmR, [128, KT], "(kt p) o -> p (kt o)")
        tmG_t = load_const(tmG, [128, KT], "(kt p) o -> p (kt o)")
        fmK_t = load_const(fmK, [128, KT], "(kt p) o -> p (kt o)")
        fmR_t = load_const(fmR, [128, KT], "(kt p) o -> p (kt o)")
        powR_t = load_const(POW_R, [64, HPL, L], "(h p) i -> p h i", p=64)
        powK_t = load_const(POW_K, [64, HPL, L], "(h p) i -> p h i", p=64)
        powU_t = load_const(POW_U, [64, HPL, L], "(h p) i -> p h i", p=64)
        powCT_t = load_const(POW_CT, [128, CHL])
        dl_t = load_const(DL, [64, HPL], "(h p) o -> p (h o)", p=64)
        maskT_t = load_const(MASKT, [128, L])
        ident_t = load_const(IDENT, [128, L])
        ones_r = load_const(ONESC, [128, 1], dt=F32R)
        ones1_r = load_const(ONESR, [1, 128], dt=F32R)
        eps_t = csts.tile([1, 1], F32)
        nc.vector.memset(eps_t, EPS)
        geps_t = csts.tile([128, 1], F32)
        nc.vector.memset(geps_t, 64.0 * EPS)

        # ---------------- shared big slots ----------------
        def new_bigA():
            # 64KB/part: xn -> ag_sb -> xn2 -> kk
            return big.tile([128, KT, T], F32R, tag="bigA", name="bigA")

        def new_mid(nfloats):
            # 48KB/part: (rT|kc|vtok) then (srec|kvsb)
            return big.tile([128, nfloats], F32R, tag="mid", name="mid")

        def load_wslab(w_ap, col0, cols):
            # 32KB/part slot shared with amask
            t = big.tile([128, KT, cols], F32R, tag="wsl", name="wsl")
            nc.sync.dma_start(
                out=t, in_=w_ap[:, col0:col0 + cols].rearrange(
                    "(kt p) m -> p kt m", p=128).bitcast(F32R))
            return t

        # ---------------- helpers ----------------
        def ln_stats(get_tile):
            """get_tile(kt, fc) -> [128,S] F32R AP -> (m_bc, r_bc)."""
            m = outs.tile([1, T], F32R, tag="lnm", name="lnm")
            sums = outs.tile([1, T], F32, tag="lnsum", name="lnsum")
            sumsq = outs.tile([1, T], F32, tag="lnsumsq", name="lnsumsq")
            for fc in range(2):
                ps_s = ps_sm.tile([1, S], F32, tag="sm", name="pss")
                ps_q = ps_sm.tile([1, S], F32, tag="sm", name="psq")
                for kt in range(KT):
                    xt_ = get_tile(kt, fc)
                    sq = rot.tile([128, S], F32R, tag="r512f", name="sq")
                    nc.scalar.activation(out=sq, in_=xt_.bitcast(F32),
                                         func=ACT.Square)
                    nc.tensor.matmul(ps_s, ones_r, xt_,
                                     start=(kt == 0), stop=(kt == KT - 1))
                    nc.tensor.matmul(ps_q, ones_r, sq,
                                     start=(kt == 0), stop=(kt == KT - 1))
                nc.any.tensor_copy(out=sums[:, fc * S:(fc + 1) * S], in_=ps_s)
                nc.any.tensor_copy(out=sumsq[:, fc * S:(fc + 1) * S], in_=ps_q)
            nc.scalar.mul(out=m, in_=sums, mul=1.0 / C)
            tmp = outs.tile([1, T], F32, tag="lnsum", name="lntmp")
            nc.vector.tensor_mul(out=tmp, in0=m.bitcast(F32),
                                 in1=m.bitcast(F32))
            nc.scalar.mul(out=sumsq, in_=sumsq, mul=1.0 / C)
            nc.vector.tensor_sub(out=tmp, in0=sumsq, in1=tmp)
            nc.scalar.activation(out=tmp, in_=tmp, func=ACT.Sqrt, bias=eps_t)
            rstd = outs.tile([1, T], F32R, tag="lnrstd", name="lnrstd")
            with nc.allow_low_precision("f32r rstd for broadcast matmul"):
                nc.vector.reciprocal(out=rstd, in_=tmp)
            m_bc = outs.tile([128, 2, S], F32, tag="lnmbc", name="lnmbc")
            r_bc = outs.tile([128, 2, S], F32, tag="lnrbc", name="lnrbc")
            for fc in range(2):
                for vec, dst in ((m, m_bc), (rstd, r_bc)):
                    ps_b = ps_sm.tile([128, S], F32, tag="sm", name="psb")
                    nc.tensor.matmul(ps_b, ones1_r,
                                     vec[:, fc * S:(fc + 1) * S],
                                     start=True, stop=True)
                    nc.any.tensor_copy(out=dst[:, fc, :], in_=ps_b)
            return m_bc, r_bc

        def lerp_into(dst, xnbuf, tm_t, kt, fc):
            """dst [128,S] F32R AP <- time-lerp of xn tokens [fc*S,(fc+1)*S)."""
            sc = tm_t[:, kt:kt + 1]
            d = rot2.tile([128, S], F32, tag="dtile", name="dt")
            if fc == 0:
                nc.vector.tensor_sub(out=d[:, :S - 1],
                                     in0=xnbuf[:, kt, 1:S].bitcast(F32),
                                     in1=xnbuf[:, kt, 0:S - 1].bitcast(F32))
                nc.vector.scalar_tensor_tensor(
                    out=dst[:, 1:S], in0=d[:, :S - 1], scalar=sc,
                    in1=xnbuf[:, kt, 0:S - 1].bitcast(F32),
                    op0=ALU.mult, op1=ALU.add)
                nc.vector.tensor_scalar_mul(
                    out=dst[:, 0:1], in0=xnbuf[:, kt, 0:1].bitcast(F32),
                    scalar1=sc)
            else:
                nc.vector.tensor_sub(out=d,
                                     in0=xnbuf[:, kt, S:T].bitcast(F32),
                                     in1=xnbuf[:, kt, S - 1:T - 1].bitcast(F32))
                nc.vector.scalar_tensor_tensor(
                    out=dst, in0=d, scalar=sc,
                    in1=xnbuf[:, kt, S - 1:T - 1].bitcast(F32),
                    op0=ALU.mult, op1=ALU.add)

        def lerp_tile(xnbuf, tm_t, kt, fc):
            t = rot.tile([128, S], F32R, tag="r512f", name="lerp")
            lerp_into(t, xnbuf, tm_t, kt, fc)
            return t

        # ---------------- LN1 ----------------
        xn = new_bigA()
        nc.sync.dma_start(
            out=xn,
            in_=xT.rearrange("(kt p) t -> p kt t", p=128).bitcast(F32R))
        m_bc, r_bc = ln_stats(lambda kt, fc: xn[:, kt, fc * S:(fc + 1) * S])
        for kt in range(KT):
            for fc in range(2):
                sl = xn[:, kt, fc * S:(fc + 1) * S]
                slf = sl.bitcast(F32)
                nc.vector.tensor_sub(out=sl, in0=slf, in1=m_bc[:, fc, :])
                nc.vector.tensor_mul(out=sl, in0=slf, in1=r_bc[:, fc, :])

        # ---------------- TimeMix matmul phases ----------------
        mid = new_mid(8 * T)
        kc_v = mid[:, 0:4 * T].rearrange("p (c l) -> p c l", c=NCH)
        vtok_v = mid[:, 4 * T:8 * T].rearrange("p (c l) -> p c l", c=NCH)

        def ch_phase(w_t, tm_t, post):
            for fc in range(2):
                pss = [ps_big.tile([128, S], F32, tag="bm", name="pbm")
                       for _ in range(4)]
                for kt in range(KT):
                    rhs = lerp_tile(xn, tm_t, kt, fc)
                    for mt in range(4):
                        nc.tensor.matmul(
                            pss[mt], w_t[:, kt, mt * 128:(mt + 1) * 128], rhs,
                            start=(kt == 0), stop=(kt == KT - 1))
                for mt in range(4):
                    post(mt, fc, pss[mt])

        def tok_phase(w_t, tm_t, post):
            for half in range(2):
                pss = [ps_big.tile([128, CHL], F32, tag="bm", name="pbm")
                       for _ in range(4)]
                for kt in range(KT):
                    rhs = lerp_tile(xn, tm_t, kt, half)
                    for q in range(4):
                        nc.tensor.matmul(
                            pss[q], rhs[:, q * 128:(q + 1) * 128],
                            w_t[:, kt, :],
                            start=(kt == 0), stop=(kt == KT - 1))
                for q in range(4):
                    post(half * 4 + q, pss[q])

        wr_t = load_wslab(Wr, 0, CHL)
        def post_r(mt, fc, ps):
            rt_tile = rot.tile([128, S], F32, tag="r512", name="ro")
            nc.any.tensor_copy(out=rt_tile, in_=ps)
            nc.sync.dma_start(
                out=rT_dram[mt * 128:(mt + 1) * 128, fc * S:(fc + 1) * S],
                in_=rt_tile)
        ch_phase(wr_t, tmR_t, post_r)

        wk_t = load_wslab(Wk, 0, CHL)
        def post_k(mt, fc, ps):
            kt_tile = rot.tile([128, S], F32, tag="r512", name="ko")
            nc.any.tensor_copy(out=kt_tile, in_=ps)
            nc.sync.dma_start(
                out=kT_dram[mt * 128:(mt + 1) * 128, fc * S:(fc + 1) * S],
                in_=kt_tile)
        ch_phase(wk_t, tmK_t, post_k)

        def post_ktok(tt, ps):
            nc.vector.tensor_mul(out=kc_v[:, tt, :], in0=ps, in1=powCT_t)
        tok_phase(wk_t, tmK_t, post_ktok)

        wv_t = load_wslab(Wv, 0, CHL)
        def post_vtok(tt, ps):
            nc.any.tensor_copy(out=vtok_v[:, tt, :], in_=ps)
        tok_phase(wv_t, tmV_t, post_vtok)

        wg_t = load_wslab(Wg, 0, CHL)
        def post_gtok(tt, ps):
            gt = rot.tile([128, CHL], F32, tag="r512", name="go")
            nc.scalar.activation(out=gt, in_=ps, func=ACT.Silu)
            nc.sync.dma_start(out=g_dram[tt * 128:(tt + 1) * 128, :], in_=gt)
        tok_phase(wg_t, tmG_t, post_gtok)

        # ---------------- WKV pass 1: A^T, dv ----------------
        amask = big.tile([128, NCH, HPL, L], F32, tag="wsl", name="amask")
        dv_sb = outs.tile([128, NCH * HPL], F32, tag="dv", name="dv")
        for c in range(NCH):
            for h in range(HPL):
                kslab = wkvp.tile([64, L], F32, tag="kslab", name="ksl")
                nc.sync.dma_start(
                    out=kslab,
                    in_=kT_dram[h * 64:(h + 1) * 64, c * L:(c + 1) * L])
                rslab = wkvp.tile([64, L], F32, tag="rslab", name="rsl")
                nc.sync.dma_start(
                    out=rslab,
                    in_=rT_dram[h * 64:(h + 1) * 64, c * L:(c + 1) * L])
                rdT = wkvp.tile([64, L], F32R, tag="rdT", name="rdT")
                nc.vector.tensor_mul(out=rdT, in0=rslab,
                                     in1=powR_t[:, h, :])
                kdT = wkvp.tile([64, L], F32R, tag="kdT", name="kdT")
                nc.vector.tensor_mul(out=kdT, in0=kslab,
                                     in1=powK_t[:, h, :])
                kdU = wkvp.tile([64, L], F32R, tag="kdU", name="kdU")
                nc.vector.tensor_mul(out=kdU, in0=kslab,
                                     in1=powU_t[:, h, :])
                ps_a = ps_sm.tile([128, L], F32, tag="sm", name="psa")
                nc.tensor.matmul(ps_a, kdT, rdT, start=True, stop=True)
                nc.vector.tensor_mul(out=amask[:, c, h, :], in0=ps_a,
                                     in1=maskT_t)
                ps_b2 = ps_sm.tile([128, L], F32, tag="sm", name="psb2")
                nc.tensor.matmul(ps_b2, kdU, rdT, start=True, stop=True)
                bd = wkvp.tile([128, L], F32, tag="bd", name="bd", bufs=2)
                nc.vector.tensor_mul(out=bd, in0=ps_b2, in1=ident_t)
                with nc.allow_low_precision("dv diag sum"):
                    nc.vector.tensor_reduce(
                        out=dv_sb[:, c * 8 + h:c * 8 + h + 1], in_=bd,
                        axis=mybir.AxisListType.X, op=ALU.add)

        # ---------------- WKV pass 2 ----------------
        spairs = {}
        for h in range(HPL):
            sp = state.tile([64, 64], F32R, tag=f"St{h}", name="sp")
            nc.sync.dma_start(out=sp, in_=ZERO64[0:64, :].bitcast(F32R))
            spairs[h] = sp
        for c in range(NCH):
            gslab = wkvp.tile([128, CHL], F32, tag="gslab", name="gsl", bufs=2)
            nc.sync.dma_start(out=gslab, in_=g_dram[c * 128:(c + 1) * 128, :])
            attg_c = wkvp.tile([128, CHL], F32, tag="attgc", name="attgc", bufs=2)
            for h in range(HPL):
                rslab = wkvp.tile([64, L], F32, tag="rslab", name="rsl2")
                nc.sync.dma_start(
                    out=rslab,
                    in_=rT_dram[h * 64:(h + 1) * 64, c * L:(c + 1) * L])
                rdT = wkvp.tile([64, L], F32R, tag="rdT", name="rdT2")
                nc.vector.tensor_mul(out=rdT, in0=rslab,
                                     in1=powR_t[:, h, :])
                afin = wkvp.tile([128, L], F32R, tag="afin", name="afin")
                nc.vector.scalar_tensor_tensor(
                    out=afin, in0=ident_t,
                    scalar=dv_sb[:, c * 8 + h:c * 8 + h + 1],
                    in1=amask[:, c, h, :],
                    op0=ALU.mult, op1=ALU.add)
                S_pair = spairs[h]
                ps_yt = ps_y.tile([128, 64], F32, tag="yt", name="psy")
                nc.tensor.matmul(ps_yt, afin,
                                 vtok_v[:, c, h * 64:(h + 1) * 64],
                                 start=True, stop=False)
                nc.tensor.matmul(ps_yt, rdT, S_pair,
                                 start=False, stop=True)
                ps_d = ps_sd.tile([64, 64], F32, tag="sd", name="psd")
                nc.tensor.matmul(ps_d,
                                 kc_v[:, c, h * 64:(h + 1) * 64],
                                 vtok_v[:, c, h * 64:(h + 1) * 64],
                                 start=True, stop=True)
                S_new = state.tile([64, 64], F32R, tag=f"St{h}",
                                   name="snew")
                nc.vector.scalar_tensor_tensor(
                    out=S_new,
                    in0=S_pair.bitcast(F32),
                    scalar=dl_t[:, h:h + 1],
                    in1=ps_d,
                    op0=ALU.mult, op1=ALU.add)
                spairs[h] = S_new
                stats = wkvp.tile([128, 6], F32, tag="bnst", name="bnst")
                nc.vector.bn_stats(out=stats, in_=ps_yt)
                mv = wkvp.tile([128, 2], F32, tag="bnmv", name="bnmv")
                nc.vector.bn_aggr(out=mv, in_=stats)
                std = wkvp.tile([128, 1], F32, tag="bnstd", name="bnstd")
                nc.scalar.activation(out=std, in_=mv[:, 1:2],
                                     func=ACT.Sqrt, bias=geps_t)
                rstd = wkvp.tile([128, 1], F32, tag="bnrstd", name="bnr")
                nc.vector.reciprocal(out=rstd, in_=std)
                an = wkvp.tile([128, 64], F32, tag="an", name="an")
                nc.vector.tensor_scalar(
                    out=an, in0=ps_yt, scalar1=mv[:, 0:1], scalar2=rstd,
                    op0=ALU.subtract, op1=ALU.mult)
                nc.vector.tensor_mul(
                    out=attg_c[:, h * 64:(h + 1) * 64], in0=an,
                    in1=gslab[:, h * 64:(h + 1) * 64])
            for ct in range(4):
                ps_t = ps_sm.tile([128, L], F32, tag="sm", name="pst")
                nc.tensor.transpose(
                    ps_t, attg_c[:, ct * 128:(ct + 1) * 128], ident_t)
                tt_ = rot.tile([128, L], F32, tag="r512", name="tro")
                nc.any.tensor_copy(out=tt_, in_=ps_t)
                nc.sync.dma_start(
                    out=cc_in[ct * 128:(ct + 1) * 128, c * L:(c + 1) * L],
                    in_=tt_)

        nc.gpsimd.collective_compute(
            "AllGather", ALU.bypass, ins=[cc_in], outs=[ag_out],
            replica_groups=GROUPS)

        # ---------------- Wo + residual -> x2out ----------------
        ag_sb = new_bigA()
        nc.sync.dma_start(
            out=ag_sb,
            in_=ag_out.rearrange("(kt p) t -> p kt t", p=128).bitcast(F32R))
        for q in range(4):
            wo_t = load_wslab(Wo, q * S, S)
            for fc in range(2):
                pss = [ps_big.tile([128, S], F32, tag="bm", name="pbm")
                       for _ in range(4)]
                for kt in range(KT):
                    for mt in range(4):
                        nc.tensor.matmul(
                            pss[mt], wo_t[:, kt, mt * 128:(mt + 1) * 128],
                            ag_sb[:, kt, fc * S:(fc + 1) * S],
                            start=(kt == 0), stop=(kt == KT - 1))
                for mt in range(4):
                    gm = q * 4 + mt
                    xres = rot.tile([128, S], F32, tag="r512", name="xres")
                    nc.sync.dma_start(
                        out=xres,
                        in_=xT[gm * 128:(gm + 1) * 128, fc * S:(fc + 1) * S])
                    x2t = rot.tile([128, S], F32, tag="r512", name="x2t")
                    nc.vector.tensor_add(out=x2t, in0=pss[mt], in1=xres)
                    nc.sync.dma_start(
                        out=x2out[gm * 128:(gm + 1) * 128,
                                  fc * S:(fc + 1) * S],
                        in_=x2t)

        # ---------------- LN2 (stream x2out) -> xn2 ----------------
        def x2_tile(kt, fc):
            t = rot.tile([128, S], F32R, tag="r512f", name="x2l")
            nc.sync.dma_start(
                out=t, in_=x2out[kt * 128:(kt + 1) * 128,
                                 fc * S:(fc + 1) * S].bitcast(F32R))
            return t

        m2_bc, r2_bc = ln_stats(x2_tile)
        xn2 = new_bigA()
        for kt in range(KT):
            for fc in range(2):
                t = x2_tile(kt, fc)
                sl = xn2[:, kt, fc * S:(fc + 1) * S]
                nc.vector.tensor_sub(out=sl, in0=t.bitcast(F32),
                                     in1=m2_bc[:, fc, :])
                nc.vector.tensor_mul(out=sl, in0=sl.bitcast(F32),
                                     in1=r2_bc[:, fc, :])

        # ---------------- ChannelMix ----------------
        srk = new_mid(8 * T)
        srec = srk[:, 0:4 * T].rearrange("p (s t) -> p s t", s=4).bitcast(F32)
        kv_sb = srk[:, 4 * T:8 * T].rearrange("p (s t) -> p s t",
                                              s=4).bitcast(F32)
        wrec_t = load_wslab(Wrec, 0, CHL)
        for fc in range(2):
            pss = [ps_big.tile([128, S], F32, tag="bm", name="pbm")
                   for _ in range(4)]
            for kt in range(KT):
                rhs = lerp_tile(xn2, fmR_t, kt, fc)
                for mt in range(4):
                    nc.tensor.matmul(
                        pss[mt], wrec_t[:, kt, mt * 128:(mt + 1) * 128], rhs,
                        start=(kt == 0), stop=(kt == KT - 1))
            for mt in range(4):
                nc.scalar.activation(out=srec[:, mt, fc * S:(fc + 1) * S],
                                     in_=pss[mt], func=ACT.Sigmoid)

        for kt in range(KT):
            for fc in range(2):
                t = rot.tile([128, S], F32R, tag="r512f", name="cko")
                lerp_into(t, xn2, fmK_t, kt, fc)
                nc.sync.dma_start(
                    out=ck_dram[kt * 128:(kt + 1) * 128, fc * S:(fc + 1) * S],
                    in_=t.bitcast(F32))

        kk = new_bigA()
        for q in range(4):
            wkey_t = load_wslab(Wkey, q * S, S)
            for fc in range(2):
                pss = [ps_big.tile([128, S], F32, tag="bm", name="pbm")
                       for _ in range(4)]
                for kt in range(KT):
                    rhs = rot.tile([128, S], F32R, tag="r512f", name="ckl")
                    nc.sync.dma_start(
                        out=rhs,
                        in_=ck_dram[kt * 128:(kt + 1) * 128,
                                    fc * S:(fc + 1) * S].bitcast(F32R))
                    for mt in range(4):
                        nc.tensor.matmul(
                            pss[mt], wkey_t[:, kt, mt * 128:(mt + 1) * 128],
                            rhs, start=(kt == 0), stop=(kt == KT - 1))
                for mt in range(4):
                    rl = rot.tile([128, S], F32, tag="r512", name="rl")
                    nc.scalar.activation(out=rl, in_=pss[mt], func=ACT.Relu)
                    nc.vector.tensor_mul(
                        out=kk[:, q * 4 + mt, fc * S:(fc + 1) * S],
                        in0=rl, in1=rl)

        for q in range(4):
            wval_t = load_wslab(Wval, q * S, S)
            for fc in range(2):
                pss = [ps_big.tile([128, S], F32, tag="bm", name="pbm")
                       for _ in range(4)]
                for kt in range(KT):
                    for mt in range(4):
                        nc.tensor.matmul(
                            pss[mt], wval_t[:, kt, mt * 128:(mt + 1) * 128],
                            kk[:, kt, fc * S:(fc + 1) * S],
                            start=(kt == 0), stop=(kt == KT - 1))
                for mt in range(4):
                    kvt = rot.tile([128, S], F32, tag="r512", name="kvo")
                    nc.any.tensor_copy(out=kvt, in_=pss[mt])
                    gm = q * 4 + mt
                    nc.sync.dma_start(
                        out=rs_in[gm * 128:(gm + 1) * 128,
                                  fc * S:(fc + 1) * S],
                        in_=kvt)
        nc.gpsimd.collective_compute(
            "ReduceScatter", ALU.add, ins=[rs_in], outs=[rs_out],
            replica_groups=GROUPS)

        nc.sync.dma_start(
            out=kv_sb, in_=rs_out.rearrange("(mt p) t -> p mt t", p=128))
        for mt in range(4):
            for fc in range(2):
                ot = rot.tile([128, S], F32, tag="r512", name="ot")
                nc.vector.tensor_mul(out=ot,
                                     in0=srec[:, mt, fc * S:(fc + 1) * S],
                                     in1=kv_sb[:, mt, fc * S:(fc + 1) * S])
                nc.sync.dma_start(
                    out=o1[mt * 128:(mt + 1) * 128, fc * S:(fc + 1) * S],
                    in_=ot)

    nc.compile()
    return nc


def _host_inputs(inputs):
    f32 = np.float32
    x = np.asarray(inputs['x'], f32)
    for k in ('ln1_g', 'ln2_g', 'lnx_g'):
        assert np.allclose(np.asarray(inputs[k]), 1.0), f"{k} not identity"
    for k in ('ln1_b', 'ln2_b', 'lnx_b'):
        assert np.allclose(np.asarray(inputs[k]), 0.0), f"{k} not zero"

    dec = np.exp(-np.exp(np.asarray(inputs['time_decay'], np.float64)))
    u = np.asarray(inputs['time_faaaa'], np.float64)
    i_idx = np.arange(L, dtype=np.float64)

    maskT = np.tril(np.ones((L, L), f32), -1).T.copy()
    ident = np.eye(L, dtype=f32)

    def cvec(a):
        return np.ascontiguousarray(np.asarray(a, f32).reshape(C, 1))

    in_maps = []
    for core in range(NCORES):
        g, lane = divmod(core, LANES)
        hsl = slice(lane * HPL, (lane + 1) * HPL)
        dlh = dec[hsl]            # [HPL, N]
        ulh = u[hsl]
        pow_r = dlh[:, None, :] ** i_idx[None, :, None]            # [HPL,L,N]
        pow_k = dlh[:, None, :] ** (-(i_idx[None, :, None] + 1))
        pow_u = ulh[:, None, :] * dlh[:, None, :] ** (-i_idx[None, :, None])
        pow_c = dlh[:, None, :] ** (L - 1 - i_idx[None, :, None])

        def chmaj(p):   # [HPL, L, N] -> [CHL, L]
            return np.ascontiguousarray(
                p.transpose(0, 2, 1).reshape(CHL, L).astype(f32))

        POW_CT = np.ascontiguousarray(
            pow_c.transpose(1, 0, 2).reshape(L, CHL).astype(f32))
        csl = slice(lane * CHL, (lane + 1) * CHL)
        ffsl = slice(lane * FFL, (lane + 1) * FFL)
        in_maps.append({
            'xT': np.ascontiguousarray(x[g].T),
            'Wr': np.ascontiguousarray(np.asarray(inputs['Wr'], f32)[:, csl]),
            'Wk': np.ascontiguousarray(np.asarray(inputs['Wk'], f32)[:, csl]),
            'Wv': np.ascontiguousarray(np.asarray(inputs['Wv'], f32)[:, csl]),
            'Wg': np.ascontiguousarray(np.asarray(inputs['Wg'], f32)[:, csl]),
            'Wo': np.ascontiguousarray(np.asarray(inputs['Wo'], f32)),
            'Wkey': np.ascontiguousarray(
                np.asarray(inputs['Wkey'], f32)[:, ffsl]),
            'Wval': np.ascontiguousarray(
                np.asarray(inputs['Wval'], f32)[ffsl, :]),
            'Wrec': np.ascontiguousarray(np.asarray(inputs['Wrec'], f32)[:, csl]),
            'tmK': cvec(inputs['tm_k']), 'tmV': cvec(inputs['tm_v']),
            'tmR': cvec(inputs['tm_r']), 'tmG': cvec(inputs['tm_g']),
            'fmK': cvec(inputs['fm_k']), 'fmR': cvec(inputs['fm_r']),
            'POW_R': chmaj(pow_r), 'POW_K': chmaj(pow_k),
            'POW_U': chmaj(pow_u), 'POW_CT': POW_CT,
            'DL': np.ascontiguousarray((dlh ** L).reshape(CHL, 1).astype(f32)),
            'MASKT': maskT, 'IDENT': ident,
            'ONESC': np.ones((128, 1), f32),
            'ONESR': np.ones((1, 128), f32),
            'ZERO64': np.zeros((128, 64), f32),
        })
    return in_maps


_LAST_RESULT = {}


def kernel(**inputs):
    global _PROGRAM
    from concourse.bass_utils import run_bass_kernel_spmd
    if _PROGRAM is None:
        _PROGRAM = _build_program()
    in_maps = _host_inputs(inputs)
    trace = bool(int(__import__('os').environ.get('KERNEL_TRACE', '0')))
    res = run_bass_kernel_spmd(_PROGRAM, in_maps, list(range(NCORES)),
                               trace=trace)
    _LAST_RESULT['res'] = res
    out = np.empty((B, T, C), np.float32)
    for core in range(NCORES):
        g, lane = divmod(core, LANES)
        r = res.results[core]
        sl = slice(lane * CHL, (lane + 1) * CHL)
        out[g, :, sl] = (r['o1'] + r['x2out'][sl, :]).T
    return out



# revision 14
# speedup vs baseline: 1.8425x; 1.8425x over previous
"""RWKV-5 block (TimeMix + ChannelMix) on 8 Trainium2 NeuronCores.

Sharding: 2 batch groups x 4-way tensor-parallel (core = 4*g + lane).
TimeMix heads split 8/lane; Wo row-sharded (lane computes its 512 output
rows); x2 AllGathered per group; ChannelMix FF split 2048/lane with kv
partials ReduceScattered. All big GEMMs run in bf16 (fp32 PSUM); the WKV
state chain is kept in fp32. Collectives are split into token halves and
pipelined against compute. Activations stay SBUF-resident (channel-major
x^T [C,T]); WKV is one fused chunk loop (L=128) using block-diagonal
head-pair matmuls; LN stats via PE ones-reduction.
Host assembles [B,T,C] from per-core o1 = x2_rows + cmix rows.
"""
import sys
import numpy as np

sys.path.insert(0, '/opt/trn_rl_repo')

B, T, C, H, N, FF = 2, 1024, 2048, 32, 64, 8192
EPS = 1e-5
L = 128            # WKV chunk length
NCH = T // L       # 8 chunks
NCORES = 8
LANES = 4
HPL = H // LANES   # 8 heads per lane
CHL = HPL * N      # 512 att channels per lane
FFL = FF // LANES  # 2048 ff channels per lane
KT = C // 128      # 16 contraction tiles
KTF = FFL // 128   # 16 ff contraction tiles
S = 512            # token half
GROUPS = [[0, 1, 2, 3], [4, 5, 6, 7]]

_PROGRAM = None


def _build_program(debug=False):
    import concourse.bacc as bacc
    import concourse.tile as tile
    from concourse import mybir
    from contextlib import ExitStack

    F32 = mybir.dt.float32
    BF16 = mybir.dt.bfloat16
    ALU = mybir.AluOpType
    ACT = mybir.ActivationFunctionType

    nc = bacc.Bacc("TRN2", target_bir_lowering=False, debug=False,
                   num_devices=NCORES)

    def din(name, shape, dt=BF16):
        return nc.dram_tensor(name, shape, dt, kind="ExternalInput").ap()

    xRES = din("xRES", [CHL, T], F32)
    xTb = din("xTb", [C, T])
    Wr = din("Wr", [C, CHL]); Wk = din("Wk", [C, CHL])
    Wv = din("Wv", [C, CHL]); Wg = din("Wg", [C, CHL])
    Wo = din("Wo", [C, CHL])
    Wrec = din("Wrec", [C, CHL])
    Wkey = din("Wkey", [C, FFL]); Wval = din("Wval", [FFL, C])
    TMK = din("TMK", [128, KT], F32); TMV = din("TMV", [128, KT], F32)
    TMR = din("TMR", [128, KT], F32); TMG = din("TMG", [128, KT], F32)
    FMK = din("FMK", [128, KT], F32); FMR = din("FMR", [128, KT], F32)
    POWR = din("POWR", [128, 4, L]); POWK = din("POWK", [128, 4, L])
    POWU = din("POWU", [128, 4, L]); POWCT = din("POWCT", [L, CHL])
    DL = din("DL", [128, 4], F32)
    MASKT2 = din("MASKT2", [128, 2 * L]); IDENT2 = din("IDENT2", [128, 2 * L])
    IDENT = din("IDENT", [128, 128])
    ONESC = din("ONESC", [128, 1]); ONESR = din("ONESR", [1, 128])

    o1 = nc.dram_tensor("o1", [CHL, T], F32, kind="ExternalOutput").ap()

    dbg = {}
    if debug:
        def dout(name, shape, dt=BF16):
            dbg[name] = nc.dram_tensor(name, shape, dt,
                                       kind="ExternalOutput").ap()
        dout("d_xn", [128, KT, T + 1])
        dout("d_rT", [128, 4, T]); dout("d_kT", [128, 4, T])
        dout("d_vtok", [128, 4, T]); dout("d_kc", [128, 4, T])
        dout("d_g", [128, NCH, CHL])
        dout("d_af0", [128, 4, 2 * L]); dout("d_y0", [128, HPL, N], F32)
        dout("d_y1", [128, HPL, N], F32)
        dout("d_S1", [128, 4 * 128], F32)
        dout("d_Sb0", [128, 4 * 128])
        dout("d_rdT1", [128, 4, L])
        dout("d_xn2", [128, KT, T + 1])
        dout("d_srec", [128, 4, T]); dout("d_ck0", [128, KT, S])
        dout("d_kk", [128, KTF, T])
        for h in range(2):
            dout(f"d_cc{h}", [CHL, S]); dout(f"d_cc2{h}", [CHL, S])
            dout(f"d_rs{h}", [CHL, S])

    cc_h = [nc.dram_tensor(f"cc_h{h}", [CHL, S], BF16).ap() for h in range(2)]
    ag_h = [nc.dram_tensor(f"ag_h{h}", [C, S], BF16).ap() for h in range(2)]
    cc2_h = [nc.dram_tensor(f"cc2_h{h}", [CHL, S], BF16).ap()
             for h in range(2)]
    ag2_h = [nc.dram_tensor(f"ag2_h{h}", [C, S], BF16).ap() for h in range(2)]
    rs_in_h = [nc.dram_tensor(f"rs_in_h{h}", [C, S], BF16).ap()
               for h in range(2)]
    rs_out_h = [nc.dram_tensor(f"rs_out_h{h}", [CHL, S], BF16).ap()
                for h in range(2)]

    with tile.TileContext(nc) as tc, ExitStack() as ctx:
        sb = ctx.enter_context(tc.tile_pool(name="sb", bufs=1))
        ps = ctx.enter_context(tc.tile_pool(name="ps", bufs=1, space="PSUM"))

        # ---------------- constants ----------------
        def load_const(ap, shape, dt=BF16, name="c"):
            t = sb.tile(shape, dt, tag=name, name=name)
            nc.sync.dma_start(out=t, in_=ap)
            return t

        tmK_t = load_const(TMK, [128, KT], F32, "tmK")
        tmV_t = load_const(TMV, [128, KT], F32, "tmV")
        tmR_t = load_const(TMR, [128, KT], F32, "tmR")
        tmG_t = load_const(TMG, [128, KT], F32, "tmG")
        fmK_t = load_const(FMK, [128, KT], F32, "fmK")
        fmR_t = load_const(FMR, [128, KT], F32, "fmR")
        powR_t = load_const(POWR, [128, 4, L], BF16, "powR")
        powK_t = load_const(POWK, [128, 4, L], BF16, "powK")
        powU_t = load_const(POWU, [128, 4, L], BF16, "powU")
        powCT_t = load_const(POWCT, [128, CHL], BF16, "powCT")
        dl_t = load_const(DL, [128, 4], F32, "dl")
        maskT2_t = load_const(MASKT2, [128, 2 * L], BF16, "maskT2")
        ident2_t = load_const(IDENT2, [128, 2 * L], BF16, "ident2")
        ident_t = load_const(IDENT, [128, 128], BF16, "ident")
        ones_c = load_const(ONESC, [128, 1], BF16, "onesc")
        ones_r = load_const(ONESR, [1, 128], BF16, "onesr")
        eps_t = sb.tile([1, 1], F32, tag="eps", name="eps")
        nc.vector.memset(eps_t, EPS)
        geps_t = sb.tile([128, 1], F32, tag="geps", name="geps")
        nc.vector.memset(geps_t, 64.0 * EPS)

        # ---------------- big persistent tiles ----------------
        # xn / xn2 share a slot (tag bigx); token index is padded by 1 so
        # the time-shift is a plain AP offset (col 0 == 0).
        def new_bigx(name):
            return sb.tile([128, KT, T + 1], BF16, tag="bigx", name=name)

        # rT/kT/vtok/kc share one 32KB slot (midA); later reused by kk.
        midA = sb.tile([128, 16, T], BF16, tag="midA", name="midA")
        rT_sb = midA[:, 0:4, :]                     # [128, 4mt, T] ch-major
        kT_sb = midA[:, 4:8, :]
        vtok = midA[:, 8:12, :].rearrange("p a (c x) -> p (a c) x", x=CHL)
        kc_sb = midA[:, 12:16, :].rearrange("p a (c x) -> p (a c) x", x=CHL)

        g_sb = sb.tile([128, NCH, CHL], BF16, tag="gsb", name="gsb")
        attg = sb.tile([128, NCH, CHL], BF16, tag="attg", name="attg")
        x2_sb = sb.tile([128, 4, T], F32, tag="x2", name="x2")
        srec = sb.tile([128, 4, T], BF16, tag="srec", name="srec")

        # WKV states: fp32 master + bf16 mirror, block-diagonal per pair.
        S_m = []
        S_b = []
        for pr in range(4):
            sm_ = sb.tile([128, 128], F32, tag=f"Sm{pr}", name=f"Sm{pr}")
            nc.vector.memset(sm_, 0.0)
            sbf = sb.tile([128, 128], BF16, tag=f"Sb{pr}", name=f"Sb{pr}")
            nc.vector.memset(sbf, 0.0)
            S_m.append(sm_)
            S_b.append(sbf)
        rhsA = []
        for pr in range(4):
            t = sb.tile([128, 2 * L], BF16, tag=f"rhsA{pr}", name=f"rhsA{pr}")
            nc.vector.memset(t, 0.0)
            rhsA.append(t)

        # ---------------- streamed weight tiles ----------------
        # One [128, cols] row-block per contraction step; bufs=4 gives the
        # DMA a few-kt prefetch lookahead across phase boundaries.
        def wtile(w_ap, kt, cols, col0=0):
            t = sb.tile([128, cols], BF16, tag="wst", name="wst", bufs=4)
            nc.sync.dma_start(
                out=t,
                in_=w_ap[kt * 128:(kt + 1) * 128, col0:col0 + cols])
            return t

        # ---------------- LN stats helper ----------------
        def ln_stats(xbuf, fcs, name):
            """Mean/rstd over channels for token halves in `fcs`.
            Returns (m_bc, r_bc) [128, 2, S] bf16 broadcast tiles."""
            m_bc = sb.tile([128, 2, S], BF16, tag="lnmbc", name=f"{name}m")
            r_bc = sb.tile([128, 2, S], BF16, tag="lnrbc", name=f"{name}r")
            for fc in fcs:
                ps_s = ps.tile([1, S], F32, tag="sm", name="pss", bufs=2)
                ps_q = ps.tile([1, S], F32, tag="sm", name="psq", bufs=2)
                for kt in range(KT):
                    xt_ = xbuf[:, kt, 1 + fc * S:1 + (fc + 1) * S]
                    sq = sb.tile([128, S], BF16, tag="lnsq", name="sq",
                                 bufs=2)
                    nc.scalar.activation(out=sq, in_=xt_, func=ACT.Square)
                    nc.tensor.matmul(ps_s, ones_c, xt_,
                                     start=(kt == 0), stop=(kt == KT - 1))
                    nc.tensor.matmul(ps_q, ones_c, sq,
                                     start=(kt == 0), stop=(kt == KT - 1))
                sums = sb.tile([1, S], F32, tag="lnsums", name="sums", bufs=2)
                m = sb.tile([1, S], F32, tag="lnm", name="m", bufs=2)
                nc.scalar.mul(out=m, in_=ps_s, mul=1.0 / C)
                nc.vector.tensor_mul(out=sums, in0=m, in1=m)
                tmp = sb.tile([1, S], F32, tag="lntmp", name="tmp", bufs=2)
                nc.scalar.mul(out=tmp, in_=ps_q, mul=1.0 / C)
                nc.vector.tensor_sub(out=tmp, in0=tmp, in1=sums)
                nc.scalar.activation(out=tmp, in_=tmp, func=ACT.Sqrt,
                                     bias=eps_t)
                rstd = sb.tile([1, S], BF16, tag="lnrstd", name="rstd",
                               bufs=2)
                with nc.allow_low_precision("bf16 rstd broadcast"):
                    nc.vector.reciprocal(out=rstd, in_=tmp)
                mb = sb.tile([1, S], BF16, tag="lnmb", name="mb", bufs=2)
                nc.vector.tensor_copy(out=mb, in_=m)
                for vec, dst in ((mb, m_bc), (rstd, r_bc)):
                    ps_b = ps.tile([128, S], F32, tag="sm", name="psb", bufs=2)
                    nc.tensor.matmul(ps_b, ones_r, vec, start=True, stop=True)
                    nc.vector.tensor_copy(out=dst[:, fc, :], in_=ps_b)
            return m_bc, r_bc

        def ln_norm(xbuf, m_bc, r_bc, fcs):
            for kt in range(KT):
                for fc in fcs:
                    sl = xbuf[:, kt, 1 + fc * S:1 + (fc + 1) * S]
                    nc.vector.tensor_sub(out=sl, in0=sl, in1=m_bc[:, fc, :])
                    nc.vector.tensor_mul(out=sl, in0=sl, in1=r_bc[:, fc, :])

        # ---------------- lerp helper ----------------
        def lerp_into(dst, xbuf, tm_t, kt, fc):
            """dst = tm*x[t] + (1-tm)*x[t-1] for tokens fc*S.. (bf16)."""
            cur = xbuf[:, kt, 1 + fc * S:1 + (fc + 1) * S]
            prv = xbuf[:, kt, fc * S:fc * S + S]
            d = sb.tile([128, S], BF16, tag="dtile", name="d", bufs=2)
            nc.vector.tensor_sub(out=d, in0=cur, in1=prv)
            nc.vector.scalar_tensor_tensor(
                out=dst, in0=d, scalar=tm_t[:, kt:kt + 1], in1=prv,
                op0=ALU.mult, op1=ALU.add)

        def lerp_tile(xbuf, tm_t, kt, fc):
            lr = sb.tile([128, S], BF16, tag="lerp", name="lr", bufs=3)
            lerp_into(lr, xbuf, tm_t, kt, fc)
            return lr

        # ---------------- LN1 ----------------
        xn = new_bigx("xn")
        nc.vector.memset(xn[:, :, 0:1], 0.0)
        nc.sync.dma_start(
            out=xn[:, :, 1:T + 1],
            in_=xTb.rearrange("(kt p) t -> p kt t", p=128))
        m1, r1 = ln_stats(xn, (0, 1), "ln1")
        ln_norm(xn, m1, r1, (0, 1))
        if debug:
            nc.sync.dma_start(out=dbg["d_xn"], in_=xn)

        # ---------------- TimeMix projections ----------------
        def ch_phase(w_ap, tm_t, post):
            # out channel-major [CHL, T]
            for fc in range(2):
                pss = [ps.tile([128, S], F32, tag="bm", name="pbm", bufs=4)
                       for _ in range(4)]
                for kt in range(KT):
                    wt = wtile(w_ap, kt, CHL)
                    lr = lerp_tile(xn, tm_t, kt, fc)
                    for mt in range(4):
                        nc.tensor.matmul(
                            pss[mt], wt[:, mt * 128:(mt + 1) * 128], lr,
                            start=(kt == 0), stop=(kt == KT - 1))
                for mt in range(4):
                    post(mt, fc, pss[mt])

        def tok_phase(w_ap, tm_t, post):
            # out token-major [T, CHL]
            for half in range(2):
                pss = [ps.tile([128, CHL], F32, tag="bm", name="pbm", bufs=4)
                       for _ in range(4)]
                for kt in range(KT):
                    wt = wtile(w_ap, kt, CHL)
                    lr = lerp_tile(xn, tm_t, kt, half)
                    for q in range(4):
                        nc.tensor.matmul(
                            pss[q], lr[:, q * 128:(q + 1) * 128], wt,
                            start=(kt == 0), stop=(kt == KT - 1))
                for q in range(4):
                    post(half * 4 + q, pss[q])

        ch_phase(Wr, tmR_t,
                 lambda mt, fc, p: nc.any.tensor_copy(
                     out=rT_sb[:, mt, fc * S:(fc + 1) * S], in_=p))
        ch_phase(Wk, tmK_t,
                 lambda mt, fc, p: nc.any.tensor_copy(
                     out=kT_sb[:, mt, fc * S:(fc + 1) * S], in_=p))

        # kc = (k token-major) * powCT via PE transpose of kT
        for mt in range(4):
            for tc_ in range(NCH):
                ps_t = ps.tile([128, 128], BF16, tag="sm", name="ptr", bufs=2)
                nc.tensor.transpose(
                    ps_t, kT_sb[:, mt, tc_ * L:(tc_ + 1) * L], ident_t)
                nc.vector.tensor_mul(
                    out=kc_sb[:, tc_, mt * 128:(mt + 1) * 128],
                    in0=ps_t, in1=powCT_t[:, mt * 128:(mt + 1) * 128])

        tok_phase(Wv, tmV_t,
                  lambda tt, p: nc.any.tensor_copy(
                      out=vtok[:, tt, :], in_=p))
        tok_phase(Wg, tmG_t,
                  lambda tt, p: nc.scalar.activation(
                      out=g_sb[:, tt, :], in_=p, func=ACT.Silu))
        if debug:
            nc.sync.dma_start(out=dbg["d_rT"], in_=rT_sb)
            nc.sync.dma_start(out=dbg["d_kT"], in_=kT_sb)
            nc.sync.dma_start(out=dbg["d_vtok"], in_=midA[:, 8:12, :])
            nc.sync.dma_start(out=dbg["d_kc"], in_=midA[:, 12:16, :])
            nc.sync.dma_start(out=dbg["d_g"], in_=g_sb)

        # ---------------- WKV fused chunk loop ----------------
        for c in range(NCH):
            rsl = rT_sb[:, :, c * L:(c + 1) * L]   # [128, 4, L]
            ksl = kT_sb[:, :, c * L:(c + 1) * L]
            rdT = sb.tile([128, 4, L], BF16, tag="rdT", name="rdT", bufs=2)
            nc.vector.tensor_mul(out=rdT, in0=rsl, in1=powR_t)
            kdT = sb.tile([128, 4, L], BF16, tag="kdT", name="kdT", bufs=2)
            nc.vector.tensor_mul(out=kdT, in0=ksl, in1=powK_t)
            kdU = sb.tile([128, 4, L], BF16, tag="kdU", name="kdU", bufs=2)
            nc.vector.tensor_mul(out=kdU, in0=ksl, in1=powU_t)

            afin = sb.tile([128, 4, 2 * L], BF16, tag="afin", name="afin",
                           bufs=2)
            for pr in range(4):
                nc.vector.tensor_copy(out=rhsA[pr][0:64, 0:L],
                                      in_=rdT[0:64, pr, :])
                nc.vector.tensor_copy(out=rhsA[pr][64:128, L:2 * L],
                                      in_=rdT[64:128, pr, :])
                psA = ps.tile([128, 2 * L], F32, tag="bm", name="psA", bufs=4)
                nc.tensor.matmul(psA, kdT[:, pr, :], rhsA[pr],
                                 start=True, stop=True)
                psB = ps.tile([128, 2 * L], F32, tag="bm", name="psB", bufs=4)
                nc.tensor.matmul(psB, kdU[:, pr, :], rhsA[pr],
                                 start=True, stop=True)
                nc.vector.tensor_mul(out=afin[:, pr, :], in0=psA,
                                     in1=maskT2_t)
                bdt = sb.tile([128, 2 * L], BF16, tag="bdt", name="bdt",
                              bufs=2)
                nc.vector.tensor_mul(out=bdt, in0=psB, in1=ident2_t)
                nc.vector.tensor_add(out=afin[:, pr, :],
                                     in0=afin[:, pr, :], in1=bdt)

            if debug and c == 0:
                nc.sync.dma_start(out=dbg["d_af0"], in_=afin)
            afv = afin.rearrange("p a (b x) -> p (a b) x", x=L)  # [128,8,L]
            ps_y = ps.tile([128, HPL, N], F32, tag="yy", name="psy", bufs=2)
            for h in range(HPL):
                nc.tensor.matmul(ps_y[:, h, :], afv[:, h, :],
                                 vtok[:, c, h * N:(h + 1) * N],
                                 start=True, stop=True,
                                 skip_group_check=True)
            y_sb = sb.tile([128, HPL, N], F32, tag="ysb", name="ysb", bufs=2)
            if c == 0:
                nc.vector.tensor_copy(out=y_sb, in_=ps_y)
            else:
                if debug and c == 1:
                    for pr in range(4):
                        nc.sync.dma_start(
                            out=dbg["d_Sb0"][:, pr * 128:(pr + 1) * 128],
                            in_=S_b[pr])
                ps_yt = ps.tile([128, HPL, N], F32, tag="sm", name="psyt",
                                bufs=2)
                for pr in range(4):
                    nc.tensor.matmul(ps_yt[:, 2 * pr:2 * pr + 2, :],
                                     rdT[:, pr, :], S_b[pr],
                                     start=True, stop=True,
                                     skip_group_check=True)
                nc.vector.tensor_copy(out=y_sb, in_=ps_y)
                nc.vector.tensor_add(out=y_sb, in0=y_sb, in1=ps_yt)

            if debug and c <= 1:
                nc.sync.dma_start(out=dbg[f"d_y{c}"], in_=y_sb)
            if debug and c == 1:
                nc.sync.dma_start(out=dbg["d_rdT1"], in_=rdT)
            # state update: S = dl * S + sum_i kc[i] v[i]
            for half4 in range(2):
                for prh in range(2):
                    pr = half4 * 2 + prh
                    ps_d = ps.tile([128, 256], F32, tag="bm", name="psd", bufs=4)
                    nc.tensor.matmul(
                        ps_d, kc_sb[:, c, pr * 128:(pr + 1) * 128],
                        vtok[:, c, half4 * 256:(half4 + 1) * 256],
                        start=True, stop=True)
                    for lo in range(2):
                        h = pr * 2 + lo
                        rr = slice(lo * 64, lo * 64 + 64)
                        cc = slice((h % 4) * 64, (h % 4) * 64 + 64)
                        nc.vector.scalar_tensor_tensor(
                            out=S_m[pr][rr, rr], in0=S_m[pr][rr, rr],
                            scalar=dl_t[rr, pr:pr + 1], in1=ps_d[rr, cc],
                            op0=ALU.mult, op1=ALU.add)
                    nc.vector.tensor_copy(out=S_b[pr], in_=S_m[pr])

            if debug and c == 1:
                for pr in range(4):
                    nc.sync.dma_start(
                        out=dbg["d_S1"][:, pr * 128:(pr + 1) * 128],
                        in_=S_m[pr])
            # GroupNorm(y) * g  -> attg
            gn_s = sb.tile([128, HPL], F32, tag="gns", name="gns", bufs=2)
            nc.vector.tensor_reduce(out=gn_s, in_=y_sb,
                                    axis=mybir.AxisListType.X, op=ALU.add)
            ysq = sb.tile([128, HPL, N], F32, tag="ysq", name="ysq", bufs=2)
            nc.scalar.activation(out=ysq, in_=y_sb, func=ACT.Square)
            gn_q = sb.tile([128, HPL], F32, tag="gnq", name="gnq", bufs=2)
            nc.vector.tensor_reduce(out=gn_q, in_=ysq,
                                    axis=mybir.AxisListType.X, op=ALU.add)
            gm = sb.tile([128, HPL], F32, tag="gnm", name="gnm", bufs=2)
            nc.scalar.mul(out=gm, in_=gn_s, mul=1.0 / N)
            msq = sb.tile([128, HPL], F32, tag="gnmsq", name="msq", bufs=2)
            nc.vector.tensor_mul(out=msq, in0=gm, in1=gm)
            var = sb.tile([128, HPL], F32, tag="gnvar", name="var", bufs=2)
            nc.vector.scalar_tensor_tensor(
                out=var, in0=gn_q, scalar=1.0 / N, in1=msq,
                op0=ALU.mult, op1=ALU.subtract)
            std = sb.tile([128, HPL], F32, tag="gnstd", name="std", bufs=2)
            nc.scalar.activation(out=std, in_=var, func=ACT.Sqrt,
                                 bias=geps_t)
            rstd = sb.tile([128, HPL], F32, tag="gnrstd", name="rstd",
                           bufs=2)
            nc.vector.reciprocal(out=rstd, in_=std)
            attn = sb.tile([128, HPL, N], BF16, tag="attn", name="attn",
                           bufs=2)
            for h in range(HPL):
                nc.vector.tensor_scalar(
                    out=attn[:, h, :], in0=y_sb[:, h, :],
                    scalar1=gm[:, h:h + 1], scalar2=rstd[:, h:h + 1],
                    op0=ALU.subtract, op1=ALU.mult)
            nc.vector.tensor_mul(out=attg[:, c, :],
                                 in0=attn.rearrange("p a b -> p (a b)"),
                                 in1=g_sb[:, c, :])

            # transpose to channel-major and stage for AllGather
            for ct in range(4):
                ps_t = ps.tile([128, 128], BF16, tag="sm", name="ptr2", bufs=2)
                nc.tensor.transpose(
                    ps_t, attg[:, c, ct * 128:(ct + 1) * 128], ident_t)
                att_t = sb.tile([128, 128], BF16, tag="att_t", name="att_t",
                                bufs=3)
                nc.any.tensor_copy(out=att_t, in_=ps_t)
                nc.sync.dma_start(
                    out=cc_h[c // 4][ct * 128:(ct + 1) * 128,
                                     (c % 4) * L:(c % 4 + 1) * L],
                    in_=att_t)
            if c == 3 or c == 7:
                if debug:
                    nc.sync.dma_start(out=dbg[f"d_cc{c // 4}"],
                                      in_=cc_h[c // 4])
                nc.gpsimd.collective_compute(
                    "AllGather", ALU.bypass, ins=[cc_h[c // 4]],
                    outs=[ag_h[c // 4]], replica_groups=GROUPS)

        # ---------------- Wo (row-sharded) + residual -> x2 ----------------
        for h in range(2):
            pss = [ps.tile([128, S], F32, tag="bm", name="pbm", bufs=4)
                   for _ in range(4)]
            for kt in range(KT):
                wt = wtile(Wo, kt, CHL)
                agt = sb.tile([128, S], BF16, tag="agt", name="agt", bufs=3)
                nc.sync.dma_start(
                    out=agt, in_=ag_h[h][kt * 128:(kt + 1) * 128, :])
                for mt in range(4):
                    nc.tensor.matmul(
                        pss[mt], wt[:, mt * 128:(mt + 1) * 128], agt,
                        start=(kt == 0), stop=(kt == KT - 1))
            for mt in range(4):
                xres = sb.tile([128, S], F32, tag="xres", name="xres",
                               bufs=2)
                nc.sync.dma_start(
                    out=xres,
                    in_=xRES[mt * 128:(mt + 1) * 128, h * S:(h + 1) * S])
                nc.vector.tensor_add(out=x2_sb[:, mt, h * S:(h + 1) * S],
                                     in0=pss[mt], in1=xres)
                x2b = sb.tile([128, S], BF16, tag="x2b", name="x2b", bufs=2)
                nc.vector.tensor_copy(
                    out=x2b, in_=x2_sb[:, mt, h * S:(h + 1) * S])
                nc.sync.dma_start(
                    out=cc2_h[h][mt * 128:(mt + 1) * 128, :], in_=x2b)
            if debug:
                nc.sync.dma_start(out=dbg[f"d_cc2{h}"], in_=cc2_h[h])
            nc.gpsimd.collective_compute(
                "AllGather", ALU.bypass, ins=[cc2_h[h]],
                outs=[ag2_h[h]], replica_groups=GROUPS)

        # ---------------- LN2 (on gathered x2) ----------------
        xn2 = new_bigx("xn2")
        nc.vector.memset(xn2[:, :, 0:1], 0.0)
        for h in range(2):
            nc.sync.dma_start(
                out=xn2[:, :, 1 + h * S:1 + (h + 1) * S],
                in_=ag2_h[h].rearrange("(kt p) t -> p kt t", p=128))
            m2, r2 = ln_stats(xn2, (h,), f"ln2{h}")
            ln_norm(xn2, m2, r2, (h,))
        if debug:
            nc.sync.dma_start(out=dbg["d_xn2"], in_=xn2)

        # ---------------- ChannelMix ----------------
        # cr -> sigmoid(cr @ Wrec) per half; ck half 0 materialized here,
        # ck half 1 between the Wkey halves (slot shared with attg).
        def cr_phase(h, make_ck):
            pss = [ps.tile([128, S], F32, tag="bm", name="pbm", bufs=4)
                   for _ in range(4)]
            ckh = sb.tile([128, KT, S], BF16, tag="attg",
                          name=f"ck{h}") if make_ck else None
            for kt in range(KT):
                wt = wtile(Wrec, kt, CHL)
                lr = lerp_tile(xn2, fmR_t, kt, h)
                for mt in range(4):
                    nc.tensor.matmul(
                        pss[mt], wt[:, mt * 128:(mt + 1) * 128], lr,
                        start=(kt == 0), stop=(kt == KT - 1))
                if make_ck:
                    lerp_into(ckh[:, kt, :], xn2, fmK_t, kt, h)
            for mt in range(4):
                nc.scalar.activation(
                    out=srec[:, mt, h * S:(h + 1) * S], in_=pss[mt],
                    func=ACT.Sigmoid)
            return ckh

        ck0 = cr_phase(0, True)
        cr_phase(1, False)
        if debug:
            nc.sync.dma_start(out=dbg["d_srec"], in_=srec)
            nc.sync.dma_start(out=dbg["d_ck0"], in_=ck0)

        # kk = relu(ck @ Wkey)^2, fc-outer so Wval/RS can pipeline per half
        kk = sb.tile([128, KTF, T], BF16, tag="midA", name="kk")
        for h in range(2):
            if h == 0:
                ckh = ck0
            else:
                ckh = sb.tile([128, KT, S], BF16, tag="attg", name="ck1")
                for kt in range(KT):
                    lerp_into(ckh[:, kt, :], xn2, fmK_t, kt, 1)
            for q in range(4):
                pss = [ps.tile([128, S], F32, tag="bm", name="pbm", bufs=4)
                       for _ in range(4)]
                for kt in range(KT):
                    wt = wtile(Wkey, kt, S, q * S)
                    for mt in range(4):
                        nc.tensor.matmul(
                            pss[mt], wt[:, mt * 128:(mt + 1) * 128],
                            ckh[:, kt, :],
                            start=(kt == 0), stop=(kt == KT - 1))
                for mt in range(4):
                    rl = sb.tile([128, S], BF16, tag="relu", name="rl",
                                 bufs=2)
                    nc.scalar.activation(out=rl, in_=pss[mt], func=ACT.Relu)
                    nc.vector.tensor_mul(
                        out=kk[:, q * 4 + mt, h * S:(h + 1) * S],
                        in0=rl, in1=rl)

        if debug:
            nc.sync.dma_start(out=dbg["d_kk"], in_=kk)
        # kv partials = kk @ Wval -> ReduceScatter per half
        for h in range(2):
            for cq in range(4):
                pss = [ps.tile([128, S], F32, tag="bm", name="pbm", bufs=4)
                       for _ in range(4)]
                for kt in range(KTF):
                    wt = wtile(Wval, kt, S, cq * S)
                    for mt in range(4):
                        nc.tensor.matmul(
                            pss[mt], wt[:, mt * 128:(mt + 1) * 128],
                            kk[:, kt, h * S:(h + 1) * S],
                            start=(kt == 0), stop=(kt == KTF - 1))
                for mt in range(4):
                    kvt = sb.tile([128, S], BF16, tag="kvt", name="kvt",
                                  bufs=3)
                    nc.any.tensor_copy(out=kvt, in_=pss[mt])
                    nc.sync.dma_start(
                        out=rs_in_h[h][(cq * 4 + mt) * 128:
                                       (cq * 4 + mt + 1) * 128, :],
                        in_=kvt)
            nc.gpsimd.collective_compute(
                "ReduceScatter", ALU.add, ins=[rs_in_h[h]],
                outs=[rs_out_h[h]], replica_groups=GROUPS)
            if debug:
                nc.sync.dma_start(out=dbg[f"d_rs{h}"], in_=rs_out_h[h])

        # ---------------- final: o1 = x2 + srec * kv ----------------
        for h in range(2):
            kv_sb = sb.tile([128, 4, S], BF16, tag="kvsb", name="kvsb",
                            bufs=1)
            nc.sync.dma_start(
                out=kv_sb,
                in_=rs_out_h[h].rearrange("(mt p) t -> p mt t", p=128))
            for mt in range(4):
                ot = sb.tile([128, S], F32, tag="ot", name="ot", bufs=2)
                nc.vector.tensor_mul(out=ot,
                                     in0=srec[:, mt, h * S:(h + 1) * S],
                                     in1=kv_sb[:, mt, :])
                nc.vector.tensor_add(out=ot, in0=ot,
                                     in1=x2_sb[:, mt, h * S:(h + 1) * S])
                nc.sync.dma_start(
                    out=o1[mt * 128:(mt + 1) * 128, h * S:(h + 1) * S],
                    in_=ot)

    nc.compile()
    return nc


def _host_inputs(inputs):
    import ml_dtypes
    f32 = np.float32
    bf16 = ml_dtypes.bfloat16
    x = np.asarray(inputs['x'], f32)
    for k in ('ln1_g', 'ln2_g', 'lnx_g'):
        assert np.allclose(np.asarray(inputs[k]), 1.0), f"{k} not identity"
    for k in ('ln1_b', 'ln2_b', 'lnx_b'):
        assert np.allclose(np.asarray(inputs[k]), 0.0), f"{k} not zero"

    dec = np.exp(-np.exp(np.asarray(inputs['time_decay'], np.float64)))
    u = np.asarray(inputs['time_faaaa'], np.float64)
    i_idx = np.arange(L, dtype=np.float64)

    maskT = np.tril(np.ones((L, L), f32), -1).T.copy()
    ident = np.eye(L, dtype=f32)

    def bf(a):
        return np.ascontiguousarray(np.asarray(a, f32).astype(bf16))

    def vec_kt(a):
        # [C] -> [128, KT] with channel c at [c % 128, c // 128]
        return np.ascontiguousarray(
            np.asarray(a, f32).reshape(-1).reshape(KT, 128).T)

    in_maps = []
    for core in range(NCORES):
        g, lane = divmod(core, LANES)
        hsl = slice(lane * HPL, (lane + 1) * HPL)
        dlh = dec[hsl]            # [HPL, N]
        ulh = u[hsl]
        pow_r = dlh[:, None, :] ** i_idx[None, :, None]            # [HPL,L,N]
        pow_k = dlh[:, None, :] ** (-(i_idx[None, :, None] + 1))
        pow_u = ulh[:, None, :] * dlh[:, None, :] ** (-i_idx[None, :, None])
        pow_c = dlh[:, None, :] ** (L - 1 - i_idx[None, :, None])

        def pair_stack(p):  # [HPL, L, N] -> [128, 4, L] pair-stacked
            chmaj = p.transpose(0, 2, 1).reshape(CHL, L)
            return np.ascontiguousarray(
                chmaj.reshape(4, 128, L).transpose(1, 0, 2).astype(bf16))

        POWCT = np.ascontiguousarray(
            pow_c.transpose(1, 0, 2).reshape(L, CHL).astype(bf16))
        DLv = np.ascontiguousarray(
            (dlh ** L).reshape(CHL).reshape(4, 128).T.astype(f32))
        csl = slice(lane * CHL, (lane + 1) * CHL)
        ffsl = slice(lane * FFL, (lane + 1) * FFL)
        xT = np.ascontiguousarray(x[g].T)
        in_maps.append({
            'xRES': np.ascontiguousarray(xT[csl]), 'xTb': bf(xT),
            'Wr': bf(np.asarray(inputs['Wr'], f32)[:, csl]),
            'Wk': bf(np.asarray(inputs['Wk'], f32)[:, csl]),
            'Wv': bf(np.asarray(inputs['Wv'], f32)[:, csl]),
            'Wg': bf(np.asarray(inputs['Wg'], f32)[:, csl]),
            'Wo': bf(np.asarray(inputs['Wo'], f32)[:, csl]),
            'Wrec': bf(np.asarray(inputs['Wrec'], f32)[:, csl]),
            'Wkey': bf(np.asarray(inputs['Wkey'], f32)[:, ffsl]),
            'Wval': bf(np.asarray(inputs['Wval'], f32)[ffsl, :]),
            'TMK': vec_kt(inputs['tm_k']), 'TMV': vec_kt(inputs['tm_v']),
            'TMR': vec_kt(inputs['tm_r']), 'TMG': vec_kt(inputs['tm_g']),
            'FMK': vec_kt(inputs['fm_k']), 'FMR': vec_kt(inputs['fm_r']),
            'POWR': pair_stack(pow_r), 'POWK': pair_stack(pow_k),
            'POWU': pair_stack(pow_u), 'POWCT': POWCT, 'DL': DLv,
            'MASKT2': bf(np.concatenate([maskT, maskT], axis=1)),
            'IDENT2': bf(np.concatenate([ident, ident], axis=1)),
            'IDENT': bf(ident),
            'ONESC': bf(np.ones((128, 1), f32)),
            'ONESR': bf(np.ones((1, 128), f32)),
        })
    return in_maps


_LAST_RESULT = {}


def kernel(**inputs):
    global _PROGRAM
    import os
    from concourse.bass_utils import run_bass_kernel_spmd
    if _PROGRAM is None:
        _PROGRAM = _build_program(
            debug=bool(int(os.environ.get('KERNEL_DEBUG', '0'))))
    in_maps = _host_inputs(inputs)
    trace = bool(int(__import__('os').environ.get('KERNEL_TRACE', '0')))
    res = run_bass_kernel_spmd(_PROGRAM, in_maps, list(range(NCORES)),
                               trace=trace)
    _LAST_RESULT['res'] = res
    out = np.empty((B, T, C), np.float32)
    for core in range(NCORES):
        g, lane = divmod(core, LANES)
        r = res.results[core]
        sl = slice(lane * CHL, (lane + 1) * CHL)
        out[g, :, sl] = r['o1'].T
    return out
